# revision 11
# baseline (speedup 1.0000x reference)
"""Trainium2 Bass kernel: single-head causal attention, data-parallel over batch.

Problem: x [4096, 64, 128] f32, Wq/Wk/Wv [128, 64] f32.
  q,k,v = x @ W*;  scores = q k^T / sqrt(128); causal softmax; out = attn @ v.

Sharding: batch 4096 -> 8 cores x 512 batches. Each core loops over 32
super-tiles of 16 batches (1024 rows of x).

Host prep: x cast to bf16 and pre-transposed to x^T [C=128, rows] per core
(plain contiguous HWDGE loads, no PE transposes); A = Wq Wk^T / sqrt(C)
folded on host.

Compact-S dataflow: P_S is COL-TILED on the PE array — per batch-pair, two
concurrent M=64 matmuls (tile_position (0,0) / (0,64)) emit only the two
valid diagonal 64x64 score blocks, stacked on partitions: sc [128, 512]
with NO cross-batch garbage. P4 is ROW+COL-TILED: per pair, two concurrent
K=64/M=64 matmuls (tile (0,0) and (64,64)).

DEEP software pipeline — iteration st emits store(st-6), A(st), B(st-2),
C(st-4) — so the serial resource cycle
  Y -> y-copy(ACT) -> P_S -> exp(ACT) -> mask(GPS) -> P4 -> norm
spreads over 4 iterations and the period is set by per-engine work, not
chain latency. exp is emitted BEFORE y-copy so it doesn't queue behind it
on ACT. y-copy is split ACT/DVE to balance the two engines. Input DMAs
fetch 4 tiles (1 MB) at a time; stores flush 2 tiles (256 KB).
"""

import numpy as np
import ml_dtypes
from contextlib import ExitStack

B, T, C, H = 4096, 64, 128, 64
N_CORES = 8
ST_B = 16                    # batches per super-tile
ROWS = ST_B * T              # 1024
B_CORE = B // N_CORES        # 512
N_ST = B_CORE // ST_B        # 32
Y_ACT = 896                  # y-copy columns on ACT (rest on DVE); 128-multiple

_cached = {}


def _build_nc():
    import concourse.bass as bass
    import concourse.mybir as mybir
    import concourse.tile as tile
    from concourse import bacc

    F32 = mybir.dt.float32
    BF16 = mybir.dt.bfloat16

    nc = bacc.Bacc("TRN2", target_bir_lowering=False, debug=False)
    x_d = nc.dram_tensor("xt", [C, B_CORE * T], BF16, kind="ExternalInput").ap()
    at_d = nc.dram_tensor("at", [C, C], BF16, kind="ExternalInput").ap()
    wv_d = nc.dram_tensor("wv", [C, H], BF16, kind="ExternalInput").ap()
    mk_d = nc.dram_tensor("mask", [128, 512], BF16, kind="ExternalInput").ap()
    o_d = nc.dram_tensor("o", [128, N_ST * 512], BF16, kind="ExternalOutput").ap()

    with tile.TileContext(nc) as tc, ExitStack() as ctx:
        sb = ctx.enter_context(tc.tile_pool(name="sb", bufs=6))
        ps = ctx.enter_context(tc.tile_pool(name="ps", bufs=1, space="PSUM"))
        cpool = ctx.enter_context(tc.tile_pool(name="const", bufs=1))

        at_sb = cpool.tile([C, C], BF16, tag="at")
        wv_sb = cpool.tile([C, H], BF16, tag="wv")
        mk_sb = cpool.tile([128, 512], BF16, tag="mk")
        nc.sync.dma_start(at_sb[:], at_d)
        nc.sync.dma_start(wv_sb[:], wv_d)
        nc.sync.dma_start(mk_sb[:], mk_d)

        # quad loads: one DMA brings 4 super-tiles (1 MB, 8KB/partition)
        LQ = 4
        xv = x_d.rearrange("p (Q n) -> Q p n", n=LQ * ROWS)
        # paired stores: one DMA flushes 2 super-tiles (256 KB)
        ov = o_d.rearrange("p (P f) -> P p f", f=1024)

        def emit_load(quad):
            xq = sb.tile([128, LQ * ROWS], BF16, tag="xq")
            nc.sync.dma_start(xq[:], xv[quad])
            return xq

        def emit_A(st, xT_sb):
            """Y + y-copy + v + v-copy for tile st."""
            cur = {"st": st, "xT_sb": xT_sb}

            # y_ps double-buffered by parity so Y(st+1) doesn't wait for
            # tile st's PSUM->SBUF copies (breaks the critical WAR cycle)
            y_ps = ps.tile([128, 1024], F32, tag=f"y{st % 2}")
            for half in range(2):
                nc.tensor.matmul(
                    y_ps[:, 512 * half:512 * half + 512],
                    at_sb[:],
                    xT_sb[:, 512 * half:512 * half + 512],
                    start=True, stop=True,
                )
            # two separate SBUF tiles so the ACT and DVE copies are
            # independent writers (no write-write serialization)
            yA = sb.tile([128, Y_ACT], BF16, tag="yA")
            yB = sb.tile([128, 1024 - Y_ACT], BF16, tag="yB")
            nc.scalar.copy(yA[:], y_ps[:, 0:Y_ACT])
            nc.vector.tensor_copy(yB[:], y_ps[:, Y_ACT:1024])
            cur["yA"], cur["yB"] = yA, yB

            v_ps = ps.tile([128, 512], F32, tag="v")
            for m in range(8):
                nc.tensor.matmul(
                    v_ps[:, 64 * m:64 * m + 64],
                    xT_sb[:, 128 * m:128 * m + 128],
                    wv_sb[:],
                    start=True, stop=True,
                )
            v_sb = sb.tile([128, 8 * 66], BF16, tag="v_sb")
            v_sb_v = v_sb[:].rearrange("p (m z) -> p m z", z=66)
            nc.vector.tensor_copy(
                v_sb_v[:, :, 0:64],
                v_ps[:].rearrange("p (m t) -> p m t", t=64),
            )
            nc.gpsimd.memset(v_sb_v[:, :, 64:65], 1.0)
            cur["v_sb"] = v_sb
            return cur

        def emit_B(p):
            """Col-tiled P_S + exp (ACT) + causal mask (GPSIMD) for tile p."""
            sc_ps = ps.tile([128, 512], F32, tag="sc", name="sc_ps")
            xT_sb = p["xT_sb"]
            for m in range(8):
                if 128 * m + 128 <= Y_ACT:
                    y_blk = p["yA"][:, 128 * m:128 * m + 128]
                else:
                    y_blk = p["yB"][:, 128 * m - Y_ACT:128 * m - Y_ACT + 128]
                nc.tensor.matmul(
                    sc_ps[0:64, 64 * m:64 * m + 64],
                    y_blk[:, 0:64],
                    xT_sb[:, 128 * m:128 * m + 64],
                    start=True, stop=True,
                )
                nc.tensor.matmul(
                    sc_ps[64:128, 64 * m:64 * m + 64],
                    y_blk[:, 64:128],
                    xT_sb[:, 128 * m + 64:128 * m + 128],
                    start=True, stop=True,
                )
            E_raw = sb.tile([128, 512], BF16, tag="Eraw", name="E_raw")
            nc.scalar.activation(
                E_raw[:], sc_ps[:], mybir.ActivationFunctionType.Exp
            )
            E_sb = sb.tile([128, 512], BF16, tag="E", name="E_sb")
            nc.gpsimd.tensor_tensor(
                out=E_sb[:], in0=E_raw[:], in1=mk_sb[:],
                op=mybir.AluOpType.mult,
            )
            p["E_sb"] = E_sb

        def emit_C(p, o2_sb):
            """Row+col-tiled P4 + normalize for tile p; store every 2 tiles."""
            st = p["st"]
            o_ps = ps.tile([128, 1024], F32, tag="o", name="o_ps")
            E_sb, v_sb = p["E_sb"], p["v_sb"]
            for m in range(8):
                off = 512 * (m // 4) + 65 * (m % 4)
                nc.tensor.matmul(
                    o_ps[0:64, off:off + 65],
                    E_sb[0:64, 64 * m:64 * m + 64],
                    v_sb[0:64, 66 * m:66 * m + 65],
                    start=True, stop=True,
                )
                nc.tensor.matmul(
                    o_ps[64:128, off:off + 65],
                    E_sb[64:128, 64 * m:64 * m + 64],
                    v_sb[64:128, 66 * m:66 * m + 65],
                    start=True, stop=True,
                )
            opsv = o_ps[:].rearrange("p (B x) -> p B x", B=2)[:, :, 0:260]
            opsb = opsv.rearrange("p B (m z) -> p B m z", z=65)
            r_sb = sb.tile([128, 8], F32, tag="r", name="r_sb")
            r_v = r_sb[:].rearrange("p (B m) -> p B m", B=2)
            nc.vector.reciprocal(r_v.unsqueeze(3), opsb[:, :, :, 64:65])
            half = st % 2
            nc.vector.tensor_tensor(
                out=o2_sb[:, 512 * half:512 * half + 512].rearrange(
                    "p (B m t) -> p B m t", B=2, t=64),
                in0=opsb[:, :, :, 0:64],
                in1=r_v.unsqueeze(3).broadcast_to((128, 2, 4, 64)),
                op=mybir.AluOpType.mult,
            )

        tiles = {}
        xq_bufs = {0: emit_load(0)}
        o2_tiles = {}
        N_ITER = N_ST + 6
        for it in range(N_ITER):
            st = it
            # store for pair (it-6)//2: data finished last iteration, so this
            # DMA never blocks the load behind it in the Sync FIFO
            j6 = it - 6
            if 0 <= j6 < N_ST and j6 % 2 == 1:
                nc.sync.dma_start(ov[j6 // 2], o2_tiles.pop(j6 // 2)[:])
            if st < N_ST and st % LQ == 0 and st // LQ + 1 < N_ST // LQ:
                xq_bufs[st // LQ + 1] = emit_load(st // LQ + 1)
            if st < N_ST:
                xq = xq_bufs[st // LQ]
                tiles[st] = emit_A(
                    st, xq[:, (st % LQ) * ROWS:(st % LQ) * ROWS + ROWS])
            if 0 <= it - 2 < N_ST:
                emit_B(tiles[it - 2])
            if 0 <= it - 4 < N_ST:
                j = it - 4
                if j % 2 == 0:
                    o2_tiles[j // 2] = sb.tile(
                        [128, 1024], BF16, tag="o2", name="o2_sb")
                emit_C(tiles[j], o2_tiles[j // 2])
                if j % 2 == 1:
                    del tiles[j - 1], tiles[j]
        # drain remaining store
        last = (N_ST - 1) // 2
        if last in o2_tiles:
            nc.sync.dma_start(ov[last], o2_tiles.pop(last)[:])

    nc.compile()
    return nc


def _host_inputs(x, Wq, Wk, Wv):
    bf = ml_dtypes.bfloat16
    at = np.ascontiguousarray((Wk @ Wq.T * (C ** -0.5)).astype(bf))
    wv_bf = np.ascontiguousarray(Wv.astype(bf))
    tri = np.triu(np.ones((T, T), dtype=np.float32))  # [s, t]: 1 if s <= t
    mask = np.ascontiguousarray(
        np.tile(np.concatenate([tri, tri], axis=0), (1, 8)).astype(bf)
    )  # [128, 512]
    in_maps = []
    for c in range(N_CORES):
        shard = x[c * B_CORE:(c + 1) * B_CORE].reshape(B_CORE * T, C)
        xt = np.ascontiguousarray(shard.T).astype(bf)  # [128, 32768]
        in_maps.append({
            "xt": xt, "at": at, "wv": wv_bf, "mask": mask,
        })
    return in_maps


def _unshard(res_list):
    outs = []
    for r in res_list:
        o = np.asarray(r["o"], dtype=np.float32)  # [128, N_ST*512]
        # o[par*64+t, st*512 + m*64 + h] -> out[(st, m, par), t, h]
        o = o.reshape(2, 64, N_ST, 8, 64).transpose(2, 3, 0, 1, 4)
        outs.append(np.ascontiguousarray(o.reshape(B_CORE, T, H)))
    return np.concatenate(outs, axis=0)


def run(x, Wq, Wk, Wv, trace=False, **run_kwargs):
    from concourse import bass_utils

    if "nc" not in _cached:
        _cached["nc"] = _build_nc()
    nc = _cached["nc"]
    in_maps = _host_inputs(np.asarray(x), np.asarray(Wq),
                           np.asarray(Wk), np.asarray(Wv))
    res = bass_utils.run_bass_kernel_spmd(
        nc, in_maps, core_ids=list(range(N_CORES)), trace=trace, **run_kwargs
    )
    return _unshard(res.results), res


def kernel(x, Wq, Wk, Wv):
    out, _ = run(x, Wq, Wk, Wv, trace=False)
    return out


# revision 12
# speedup vs baseline: 1.0095x; 1.0095x over previous
"""Trainium2 Bass kernel: single-head causal attention, data-parallel over batch.

Problem: x [4096, 64, 128] f32, Wq/Wk/Wv [128, 64] f32.
  q,k,v = x @ W*;  scores = q k^T / sqrt(128); causal softmax; out = attn @ v.

Sharding: batch 4096 -> 8 cores x 512 batches. Each core loops over 32
super-tiles of 16 batches (1024 rows of x).

Host prep: x cast to bf16 and pre-transposed to x^T [C=128, rows] per core
(plain contiguous HWDGE loads, no PE transposes); A = Wq Wk^T / sqrt(C)
folded on host.

Compact-S dataflow: P_S is COL-TILED on the PE array — per batch-pair, two
concurrent M=64 matmuls (tile_position (0,0) / (0,64)) emit only the two
valid diagonal 64x64 score blocks, stacked on partitions: sc [128, 512]
with NO cross-batch garbage. P4 is ROW+COL-TILED: per pair, two concurrent
K=64/M=64 matmuls (tile (0,0) and (64,64)).

DEEP software pipeline — iteration st emits store(st-6), A(st), B(st-2),
C(st-4) — so the serial resource cycle
  Y -> y-copy(ACT) -> P_S -> exp(ACT) -> mask(GPS) -> P4 -> norm
spreads over 4 iterations and the period is set by per-engine work, not
chain latency. exp is emitted BEFORE y-copy so it doesn't queue behind it
on ACT. y-copy is split ACT/DVE to balance the two engines. Input DMAs
fetch 4 tiles (1 MB) at a time; stores flush 2 tiles (256 KB).
"""

import numpy as np
import ml_dtypes
from contextlib import ExitStack

B, T, C, H = 4096, 64, 128, 64
N_CORES = 8
ST_B = 16                    # batches per super-tile
ROWS = ST_B * T              # 1024
B_CORE = B // N_CORES        # 512
N_ST = B_CORE // ST_B        # 32
Y_ACT = 896                  # y-copy columns on ACT (rest on DVE); 128-multiple

_cached = {}


def _build_nc():
    import concourse.bass as bass
    import concourse.mybir as mybir
    import concourse.tile as tile
    from concourse import bacc

    F32 = mybir.dt.float32
    BF16 = mybir.dt.bfloat16

    nc = bacc.Bacc("TRN2", target_bir_lowering=False, debug=False)
    x_d = nc.dram_tensor("xt", [C, B_CORE * T], BF16, kind="ExternalInput").ap()
    at_d = nc.dram_tensor("at", [C, C], BF16, kind="ExternalInput").ap()
    wv_d = nc.dram_tensor("wv", [C, H], BF16, kind="ExternalInput").ap()
    mk_d = nc.dram_tensor("mask", [128, 512], BF16, kind="ExternalInput").ap()
    o_d = nc.dram_tensor("o", [128, N_ST * 512], BF16, kind="ExternalOutput").ap()

    with tile.TileContext(nc) as tc, ExitStack() as ctx:
        sb = ctx.enter_context(tc.tile_pool(name="sb", bufs=6))
        ps = ctx.enter_context(tc.tile_pool(name="ps", bufs=1, space="PSUM"))
        cpool = ctx.enter_context(tc.tile_pool(name="const", bufs=1))

        at_sb = cpool.tile([C, C], BF16, tag="at")
        wv_sb = cpool.tile([C, H], BF16, tag="wv")
        mk_sb = cpool.tile([128, 512], BF16, tag="mk")
        nc.sync.dma_start(at_sb[:], at_d)
        nc.sync.dma_start(wv_sb[:], wv_d)
        nc.sync.dma_start(mk_sb[:], mk_d)

        # quad loads: one DMA brings 4 super-tiles (1 MB, 8KB/partition)
        LQ = 4
        xv = x_d.rearrange("p (Q n) -> Q p n", n=LQ * ROWS)
        # paired stores: one DMA flushes 2 super-tiles (256 KB)
        ov = o_d.rearrange("p (P f) -> P p f", f=1024)

        def emit_load(quad):
            xq = sb.tile([128, LQ * ROWS], BF16, tag="xq")
            nc.sync.dma_start(xq[:], xv[quad])
            return xq

        def emit_A(st, xT_sb):
            """Y + y-copy + v + v-copy for tile st."""
            cur = {"st": st, "xT_sb": xT_sb}

            # y_ps double-buffered by parity so Y(st+1) doesn't wait for
            # tile st's PSUM->SBUF copies (breaks the critical WAR cycle)
            y_ps = ps.tile([128, 1024], F32, tag=f"y{st % 2}")
            for half in range(2):
                nc.tensor.matmul(
                    y_ps[:, 512 * half:512 * half + 512],
                    at_sb[:],
                    xT_sb[:, 512 * half:512 * half + 512],
                    start=True, stop=True,
                )
            # two separate SBUF tiles so the ACT and DVE copies are
            # independent writers (no write-write serialization)
            yA = sb.tile([128, Y_ACT], BF16, tag="yA")
            yB = sb.tile([128, 1024 - Y_ACT], BF16, tag="yB")
            nc.scalar.copy(yA[:], y_ps[:, 0:Y_ACT])
            nc.vector.tensor_copy(yB[:], y_ps[:, Y_ACT:1024])
            cur["yA"], cur["yB"] = yA, yB

            v_ps = ps.tile([128, 512], F32, tag="v")
            for m in range(8):
                nc.tensor.matmul(
                    v_ps[:, 64 * m:64 * m + 64],
                    xT_sb[:, 128 * m:128 * m + 128],
                    wv_sb[:],
                    start=True, stop=True,
                )
            v_sb = sb.tile([128, 8 * 66], BF16, tag="v_sb")
            v_sb_v = v_sb[:].rearrange("p (m z) -> p m z", z=66)
            nc.vector.tensor_copy(
                v_sb_v[:, :, 0:64],
                v_ps[:].rearrange("p (m t) -> p m t", t=64),
            )
            nc.gpsimd.memset(v_sb_v[:, :, 64:65], 1.0)
            cur["v_sb"] = v_sb
            return cur

        def emit_B(p):
            """Col-tiled P_S + exp (ACT) + causal mask (GPSIMD) for tile p."""
            sc_ps = ps.tile([128, 512], F32, tag="sc", name="sc_ps")
            xT_sb = p["xT_sb"]
            for m in range(8):
                if 128 * m + 128 <= Y_ACT:
                    y_blk = p["yA"][:, 128 * m:128 * m + 128]
                else:
                    y_blk = p["yB"][:, 128 * m - Y_ACT:128 * m - Y_ACT + 128]
                nc.tensor.matmul(
                    sc_ps[0:64, 64 * m:64 * m + 64],
                    y_blk[:, 0:64],
                    xT_sb[:, 128 * m:128 * m + 64],
                    start=True, stop=True,
                )
                nc.tensor.matmul(
                    sc_ps[64:128, 64 * m:64 * m + 64],
                    y_blk[:, 64:128],
                    xT_sb[:, 128 * m + 64:128 * m + 128],
                    start=True, stop=True,
                )
            E_raw = sb.tile([128, 512], BF16, tag="Eraw", name="E_raw")
            nc.scalar.activation(
                E_raw[:], sc_ps[:], mybir.ActivationFunctionType.Exp
            )
            E_sb = sb.tile([128, 512], BF16, tag="E", name="E_sb")
            nc.gpsimd.tensor_tensor(
                out=E_sb[:], in0=E_raw[:], in1=mk_sb[:],
                op=mybir.AluOpType.mult,
            )
            p["E_sb"] = E_sb

        def emit_C(p, o2_sb):
            """Row+col-tiled P4 + normalize for tile p; store every 2 tiles."""
            st = p["st"]
            o_ps = ps.tile([128, 1024], F32, tag="o", name="o_ps")
            E_sb, v_sb = p["E_sb"], p["v_sb"]
            for m in range(8):
                off = 512 * (m // 4) + 65 * (m % 4)
                nc.tensor.matmul(
                    o_ps[0:64, off:off + 65],
                    E_sb[0:64, 64 * m:64 * m + 64],
                    v_sb[0:64, 66 * m:66 * m + 65],
                    start=True, stop=True,
                )
                nc.tensor.matmul(
                    o_ps[64:128, off:off + 65],
                    E_sb[64:128, 64 * m:64 * m + 64],
                    v_sb[64:128, 66 * m:66 * m + 65],
                    start=True, stop=True,
                )
            opsv = o_ps[:].rearrange("p (B x) -> p B x", B=2)[:, :, 0:260]
            opsb = opsv.rearrange("p B (m z) -> p B m z", z=65)
            r_sb = sb.tile([128, 8], F32, tag="r", name="r_sb")
            r_v = r_sb[:].rearrange("p (B m) -> p B m", B=2)
            nc.vector.reciprocal(r_v.unsqueeze(3), opsb[:, :, :, 64:65])
            half = st % 2
            nc.vector.tensor_tensor(
                out=o2_sb[:, 512 * half:512 * half + 512].rearrange(
                    "p (B m t) -> p B m t", B=2, t=64),
                in0=opsb[:, :, :, 0:64],
                in1=r_v.unsqueeze(3).broadcast_to((128, 2, 4, 64)),
                op=mybir.AluOpType.mult,
            )

        tiles = {}
        xq_bufs = {0: emit_load(0)}
        o2_tiles = {}
        N_ITER = N_ST + 6
        for it in range(N_ITER):
            st = it
            # store for pair (it-6)//2: data finished last iteration, so this
            # DMA never blocks the load behind it in the Sync FIFO
            j6 = it - 6
            if 0 <= j6 < N_ST and j6 % 2 == 1:
                nc.sync.dma_start(ov[j6 // 2], o2_tiles.pop(j6 // 2)[:])
            if st < N_ST and st % LQ == 0 and st // LQ + 1 < N_ST // LQ:
                xq_bufs[st // LQ + 1] = emit_load(st // LQ + 1)
            if 0 <= it - 2 < N_ST:
                emit_B(tiles[it - 2])
            if 0 <= it - 4 < N_ST:
                j = it - 4
                if j % 2 == 0:
                    o2_tiles[j // 2] = sb.tile(
                        [128, 1024], BF16, tag="o2", name="o2_sb")
                emit_C(tiles[j], o2_tiles[j // 2])
                if j % 2 == 1:
                    del tiles[j - 1], tiles[j]
            if st < N_ST:
                xq = xq_bufs[st // LQ]
                tiles[st] = emit_A(
                    st, xq[:, (st % LQ) * ROWS:(st % LQ) * ROWS + ROWS])
        # drain remaining store
        last = (N_ST - 1) // 2
        if last in o2_tiles:
            nc.sync.dma_start(ov[last], o2_tiles.pop(last)[:])

    nc.compile()
    return nc


def _host_inputs(x, Wq, Wk, Wv):
    bf = ml_dtypes.bfloat16
    at = np.ascontiguousarray((Wk @ Wq.T * (C ** -0.5)).astype(bf))
    wv_bf = np.ascontiguousarray(Wv.astype(bf))
    tri = np.triu(np.ones((T, T), dtype=np.float32))  # [s, t]: 1 if s <= t
    mask = np.ascontiguousarray(
        np.tile(np.concatenate([tri, tri], axis=0), (1, 8)).astype(bf)
    )  # [128, 512]
    in_maps = []
    for c in range(N_CORES):
        shard = x[c * B_CORE:(c + 1) * B_CORE].reshape(B_CORE * T, C)
        xt = np.ascontiguousarray(shard.T).astype(bf)  # [128, 32768]
        in_maps.append({
            "xt": xt, "at": at, "wv": wv_bf, "mask": mask,
        })
    return in_maps


def _unshard(res_list):
    outs = []
    for r in res_list:
        o = np.asarray(r["o"], dtype=np.float32)  # [128, N_ST*512]
        # o[par*64+t, st*512 + m*64 + h] -> out[(st, m, par), t, h]
        o = o.reshape(2, 64, N_ST, 8, 64).transpose(2, 3, 0, 1, 4)
        outs.append(np.ascontiguousarray(o.reshape(B_CORE, T, H)))
    return np.concatenate(outs, axis=0)


def run(x, Wq, Wk, Wv, trace=False, **run_kwargs):
    from concourse import bass_utils

    if "nc" not in _cached:
        _cached["nc"] = _build_nc()
    nc = _cached["nc"]
    in_maps = _host_inputs(np.asarray(x), np.asarray(Wq),
                           np.asarray(Wk), np.asarray(Wv))
    res = bass_utils.run_bass_kernel_spmd(
        nc, in_maps, core_ids=list(range(N_CORES)), trace=trace, **run_kwargs
    )
    return _unshard(res.results), res


def kernel(x, Wq, Wk, Wv):
    out, _ = run(x, Wq, Wk, Wv, trace=False)
    return out


# revision 13
# speedup vs baseline: 1.0494x; 1.0395x over previous
"""Trainium2 Bass kernel: single-head causal attention, data-parallel over batch.

Problem: x [4096, 64, 128] f32, Wq/Wk/Wv [128, 64] f32.
  q,k,v = x @ W*;  scores = q k^T / sqrt(128); causal softmax; out = attn @ v.

Sharding: batch 4096 -> 8 cores x 512 batches. Each core loops over 32
super-tiles of 16 batches (1024 rows of x).

Host prep: x cast to bf16 and pre-transposed to x^T [C=128, rows] per core
(plain contiguous HWDGE loads, no PE transposes); A = Wq Wk^T / sqrt(C)
folded on host.

Compact-S dataflow: P_S is COL-TILED on the PE array — per batch-pair, two
concurrent M=64 matmuls (tile_position (0,0) / (0,64)) emit only the two
valid diagonal 64x64 score blocks, stacked on partitions: sc [128, 512]
with NO cross-batch garbage. P4 is ROW+COL-TILED: per pair, two concurrent
K=64/M=64 matmuls (tile (0,0) and (64,64)).

DEEP software pipeline — iteration st emits store(st-6), A(st), B(st-2),
C(st-4) — so the serial resource cycle
  Y -> y-copy(ACT) -> P_S -> exp(ACT) -> mask(GPS) -> P4 -> norm
spreads over 4 iterations and the period is set by per-engine work, not
chain latency. exp is emitted BEFORE y-copy so it doesn't queue behind it
on ACT. y-copy is split ACT/DVE to balance the two engines. Input DMAs
fetch 4 tiles (1 MB) at a time; stores flush 2 tiles (256 KB).
"""

import numpy as np
import ml_dtypes
from contextlib import ExitStack

B, T, C, H = 4096, 64, 128, 64
N_CORES = 8
ST_B = 16                    # batches per super-tile
ROWS = ST_B * T              # 1024
B_CORE = B // N_CORES        # 512
N_ST = B_CORE // ST_B        # 32
Y_ACT = 1024                 # y-copy columns on ACT (rest on DVE); 128-multiple

_cached = {}


def _build_nc():
    import concourse.bass as bass
    import concourse.mybir as mybir
    import concourse.tile as tile
    from concourse import bacc

    F32 = mybir.dt.float32
    BF16 = mybir.dt.bfloat16

    nc = bacc.Bacc("TRN2", target_bir_lowering=False, debug=False)
    x_d = nc.dram_tensor("xt", [C, B_CORE * T], BF16, kind="ExternalInput").ap()
    at_d = nc.dram_tensor("at", [C, C], BF16, kind="ExternalInput").ap()
    wv_d = nc.dram_tensor("wv", [C, H], BF16, kind="ExternalInput").ap()
    mk_d = nc.dram_tensor("mask", [128, 512], BF16, kind="ExternalInput").ap()
    o_d = nc.dram_tensor("o", [128, N_ST * 512], BF16, kind="ExternalOutput").ap()

    with tile.TileContext(nc) as tc, ExitStack() as ctx:
        sb = ctx.enter_context(tc.tile_pool(name="sb", bufs=8))
        ps = ctx.enter_context(tc.tile_pool(name="ps", bufs=1, space="PSUM"))
        cpool = ctx.enter_context(tc.tile_pool(name="const", bufs=1))

        at_sb = cpool.tile([C, C], BF16, tag="at")
        wv_sb = cpool.tile([C, H], BF16, tag="wv")
        mk_sb = cpool.tile([128, 512], BF16, tag="mk")
        nc.sync.dma_start(at_sb[:], at_d)
        nc.sync.dma_start(wv_sb[:], wv_d)
        nc.sync.dma_start(mk_sb[:], mk_d)

        # pre-set the ones columns of all v_sb ring buffers ONCE: they are
        # constant, so the per-tile gpsimd memset (and its write-write
        # serialization against the DVE v-copy) disappears from the loop
        for _k in range(8):
            _vb = sb.tile([128, 8 * 66], BF16, tag="v_sb")
            nc.gpsimd.memset(
                _vb[:].rearrange("p (m z) -> p m z", z=66)[:, :, 64:65], 1.0)

        # quad loads: one DMA brings 4 super-tiles (1 MB, 8KB/partition)
        LQ = 4
        xv = x_d.rearrange("p (Q n) -> Q p n", n=LQ * ROWS)
        # paired stores: one DMA flushes 2 super-tiles (256 KB)
        ov = o_d.rearrange("p (P f) -> P p f", f=1024)

        def emit_load(quad):
            xq = sb.tile([128, LQ * ROWS], BF16, tag="xq")
            nc.sync.dma_start(xq[:], xv[quad])
            return xq

        def emit_A(st, xT_sb):
            """Y + y-copy + v + v-copy for tile st."""
            cur = {"st": st, "xT_sb": xT_sb}

            # y_ps double-buffered by parity so Y(st+1) doesn't wait for
            # tile st's PSUM->SBUF copies (breaks the critical WAR cycle)
            y_ps = ps.tile([128, 1024], F32, tag=f"y{st % 2}")
            for half in range(2):
                nc.tensor.matmul(
                    y_ps[:, 512 * half:512 * half + 512],
                    at_sb[:],
                    xT_sb[:, 512 * half:512 * half + 512],
                    start=True, stop=True,
                )
            yA = sb.tile([128, Y_ACT], BF16, tag="yA")
            nc.scalar.copy(yA[:], y_ps[:])
            cur["yA"] = yA

            v_ps = ps.tile([128, 512], F32, tag="v")
            for m in range(8):
                nc.tensor.matmul(
                    v_ps[:, 64 * m:64 * m + 64],
                    xT_sb[:, 128 * m:128 * m + 128],
                    wv_sb[:],
                    start=True, stop=True,
                )
            v_sb = sb.tile([128, 8 * 66], BF16, tag="v_sb")
            v_sb_v = v_sb[:].rearrange("p (m z) -> p m z", z=66)
            nc.vector.tensor_copy(
                v_sb_v[:, :, 0:64],
                v_ps[:].rearrange("p (m t) -> p m t", t=64),
            )
            cur["v_sb"] = v_sb
            return cur

        def emit_B(p):
            """Col-tiled P_S + exp (ACT) + causal mask (GPSIMD) for tile p."""
            sc_ps = ps.tile([128, 512], F32, tag="sc", name="sc_ps")
            xT_sb = p["xT_sb"]
            for m in range(8):
                y_blk = p["yA"][:, 128 * m:128 * m + 128]
                nc.tensor.matmul(
                    sc_ps[0:64, 64 * m:64 * m + 64],
                    y_blk[:, 0:64],
                    xT_sb[:, 128 * m:128 * m + 64],
                    start=True, stop=True,
                )
                nc.tensor.matmul(
                    sc_ps[64:128, 64 * m:64 * m + 64],
                    y_blk[:, 64:128],
                    xT_sb[:, 128 * m + 64:128 * m + 128],
                    start=True, stop=True,
                )
            E_raw = sb.tile([128, 512], BF16, tag="Eraw", name="E_raw")
            nc.scalar.activation(
                E_raw[:], sc_ps[:], mybir.ActivationFunctionType.Exp
            )
            E_sb = sb.tile([128, 512], BF16, tag="E", name="E_sb")
            nc.gpsimd.tensor_tensor(
                out=E_sb[:], in0=E_raw[:], in1=mk_sb[:],
                op=mybir.AluOpType.mult,
            )
            p["E_sb"] = E_sb

        def emit_C(p, o2_sb):
            """Row+col-tiled P4 + normalize for tile p; store every 2 tiles."""
            st = p["st"]
            o_ps = ps.tile([128, 1024], F32, tag="o", name="o_ps")
            E_sb, v_sb = p["E_sb"], p["v_sb"]
            for m in range(8):
                off = 512 * (m // 4) + 65 * (m % 4)
                nc.tensor.matmul(
                    o_ps[0:64, off:off + 65],
                    E_sb[0:64, 64 * m:64 * m + 64],
                    v_sb[0:64, 66 * m:66 * m + 65],
                    start=True, stop=True,
                )
                nc.tensor.matmul(
                    o_ps[64:128, off:off + 65],
                    E_sb[64:128, 64 * m:64 * m + 64],
                    v_sb[64:128, 66 * m:66 * m + 65],
                    start=True, stop=True,
                )
            opsv = o_ps[:].rearrange("p (B x) -> p B x", B=2)[:, :, 0:260]
            opsb = opsv.rearrange("p B (m z) -> p B m z", z=65)
            r_sb = sb.tile([128, 8], F32, tag="r", name="r_sb")
            r_v = r_sb[:].rearrange("p (B m) -> p B m", B=2)
            nc.vector.reciprocal(r_v.unsqueeze(3), opsb[:, :, :, 64:65])
            half = st % 2
            nc.vector.tensor_tensor(
                out=o2_sb[:, 512 * half:512 * half + 512].rearrange(
                    "p (B m t) -> p B m t", B=2, t=64),
                in0=opsb[:, :, :, 0:64],
                in1=r_v.unsqueeze(3).broadcast_to((128, 2, 4, 64)),
                op=mybir.AluOpType.mult,
            )

        tiles = {}
        xq_bufs = {0: emit_load(0)}
        o2_tiles = {}
        N_ITER = N_ST + 6
        for it in range(N_ITER):
            st = it
            # store for pair (it-6)//2: data finished last iteration, so this
            # DMA never blocks the load behind it in the Sync FIFO
            j6 = it - 6
            if 0 <= j6 < N_ST and j6 % 2 == 1:
                nc.sync.dma_start(ov[j6 // 2], o2_tiles.pop(j6 // 2)[:])
            if st < N_ST and st % LQ == 0 and st // LQ + 1 < N_ST // LQ:
                xq_bufs[st // LQ + 1] = emit_load(st // LQ + 1)
            if 0 <= it - 2 < N_ST:
                emit_B(tiles[it - 2])
            if 0 <= it - 4 < N_ST:
                j = it - 4
                if j % 2 == 0:
                    o2_tiles[j // 2] = sb.tile(
                        [128, 1024], BF16, tag="o2", name="o2_sb")
                emit_C(tiles[j], o2_tiles[j // 2])
                if j % 2 == 1:
                    del tiles[j - 1], tiles[j]
            if st < N_ST:
                xq = xq_bufs[st // LQ]
                tiles[st] = emit_A(
                    st, xq[:, (st % LQ) * ROWS:(st % LQ) * ROWS + ROWS])
        # drain remaining store
        last = (N_ST - 1) // 2
        if last in o2_tiles:
            nc.sync.dma_start(ov[last], o2_tiles.pop(last)[:])

    nc.compile()
    return nc


def _host_inputs(x, Wq, Wk, Wv):
    bf = ml_dtypes.bfloat16
    at = np.ascontiguousarray((Wk @ Wq.T * (C ** -0.5)).astype(bf))
    wv_bf = np.ascontiguousarray(Wv.astype(bf))
    tri = np.triu(np.ones((T, T), dtype=np.float32))  # [s, t]: 1 if s <= t
    mask = np.ascontiguousarray(
        np.tile(np.concatenate([tri, tri], axis=0), (1, 8)).astype(bf)
    )  # [128, 512]
    in_maps = []
    for c in range(N_CORES):
        shard = x[c * B_CORE:(c + 1) * B_CORE].reshape(B_CORE * T, C)
        xt = np.ascontiguousarray(shard.T).astype(bf)  # [128, 32768]
        in_maps.append({
            "xt": xt, "at": at, "wv": wv_bf, "mask": mask,
        })
    return in_maps


def _unshard(res_list):
    outs = []
    for r in res_list:
        o = np.asarray(r["o"], dtype=np.float32)  # [128, N_ST*512]
        # o[par*64+t, st*512 + m*64 + h] -> out[(st, m, par), t, h]
        o = o.reshape(2, 64, N_ST, 8, 64).transpose(2, 3, 0, 1, 4)
        outs.append(np.ascontiguousarray(o.reshape(B_CORE, T, H)))
    return np.concatenate(outs, axis=0)


def run(x, Wq, Wk, Wv, trace=False, **run_kwargs):
    from concourse import bass_utils

    if "nc" not in _cached:
        _cached["nc"] = _build_nc()
    nc = _cached["nc"]
    in_maps = _host_inputs(np.asarray(x), np.asarray(Wq),
                           np.asarray(Wk), np.asarray(Wv))
    res = bass_utils.run_bass_kernel_spmd(
        nc, in_maps, core_ids=list(range(N_CORES)), trace=trace, **run_kwargs
    )
    return _unshard(res.results), res


def kernel(x, Wq, Wk, Wv):
    out, _ = run(x, Wq, Wk, Wv, trace=False)
    return out
